# revision 37
# baseline (speedup 1.0000x reference)
"""AddAttention (Bahdanau additive attention) Trainium2 kernel, v5.

Math (per batch b):
    q   = query @ Wq + bq                          [D]
    k_t = value[t] @ Wk + bk                       [T, D]
    s_t = sum_d scale[d] * tanh(q[d] + k_t[d])     [T]
    a   = softmax(s masked to t < value_lens[b])
    out = sum_t a_t * value[t]                     [DV]

Distribution: pure data-parallel over batch B=32 across 8 NeuronCores
(4 batches per core, params replicated, no collectives).  Batches sorted
by value_lens; slot s on every core holds a batch of similar length so
the SPMD graph only processes ceil(max_len_in_slot/128) chunks per slot.

v5 design notes (on top of the v3/v4 hybrid split-K DoubleRow):
  - kT layout: kT[d, t'] = sum_j Wk[j, d]*value[t', j]; d on partitions
    so q(+bq+bk) is a per-partition tanh bias (host-computed q) and
    scale is the stationary of a score-reduce matmul.
  - SPLIT-K PRECISION HYBRID: j in [0,256) bf16 (2 matmuls), j in
    [256,512) ONE fp8e4m3 DoubleRow matmul (2 rows/cycle).  12R instead
    of 16R kproj cycles at rel-err ~1.6e-2 (gate 2e-2; full-fp8 is 2.3e-2).
  - EVERY dma_start costs ~2.2us of completion latency, serialized per
    queue (measured; the doc's `dma_us ~= 2.0 + bytes/436e3`).  So the
    startup-critical params (Wk halves, smalls) AND slot0-piece0 travel
    as ONE byte-packed "boot" DMA, consumed through .bitcast() APs.
  - slot0 pieces: piece1 alone + pieces2..3 merged, byte-packed, on the
    SCALAR queue (overlaps the gpsimd queue's fixed costs); V loads
    follow on the same scalar queue, whose serialized order keeps them
    from flooding HBM while the boot/VT transfers are critical.  Slots
    1..3 travel as ONE merged byte blob each on the gpsimd queue.
  - V loads are packed so each partition is ONE contiguous dram run
    (descriptor-count-bound otherwise).
  - warmup matmuls ramp the PE clock (an idle gap RESETS the ramp and
    the next ~6us of matmuls run at 1-1.2GHz - measured).
  - tails (softmax+context) deferred two pieces into the next slot; the
    LAST slot's score transpose runs as 4 tiny PE transpose-matmuls
    instead of a sync-queue DMA round trip (saves ~2.2us exposed).
"""

import math
from contextlib import ExitStack

import ml_dtypes
import numpy as np

import concourse.bass as bass
import concourse.bacc as bacc
import concourse.tile as tile
from concourse import mybir
from concourse import bass_utils

F32 = mybir.dt.float32
BF16 = mybir.dt.bfloat16
FP16 = mybir.dt.float16
FP8E4 = mybir.dt.float8e4
U8 = mybir.dt.uint8
I32 = mybir.dt.int32
AF = mybir.ActivationFunctionType
ALU = mybir.AluOpType
DR = mybir.MatmulPerfMode.DoubleRow

N_CORES = 8
B, TV, DQ, DV, D = 32, 2048, 512, 512, 512
SLOTS = B // N_CORES  # 4 batches per core
P = 128
KC = D // P  # 4 contraction chunks of 128
WARMUP = 19  # x [128,256] matmuls; sized to end when the boot DMA lands

BF16_NP = ml_dtypes.bfloat16
E4_NP = ml_dtypes.float8_e4m3fn
F16_NP = np.float16

# boot blob byte offsets (per partition)
BOOT_WKBF = 0                      # [2, 512] bf16   = 2048 B
BOOT_WK8 = 2048                    # [2, 512] e4m3   = 1024 B
BOOT_SMALLS = 3072                 # [24] f32        = 96 B
BOOT_BYTES = 3168                  # params only; slot0-p0 rides scalar


def _piece_widths(nch):
    """Slot of nch 128-row chunks -> pieces of <=4 chunks (512 t'-cols)."""
    out = []
    rem = nch
    while rem > 0:
        w = min(4, rem)
        out.append(w)
        rem -= w
    return out


def _nat_index(nch):
    """Row order of the natural-layout value pack: position p*nch + c
    (partition-major) holds packed row t' = 512q + p*w_q + c2."""
    idx = np.empty((P, nch), np.int64)
    for q, w in enumerate(_piece_widths(nch)):
        pcol = np.arange(P)[:, None] * w
        idx[:, 4 * q:4 * q + w] = 512 * q + pcol + np.arange(w)[None, :]
    return idx.reshape(-1)


def build_graph(nchunks):
    """Per-core Bass graph. nchunks[s] = 128-row chunks for slot s
    (descending; same on every core -> same SPMD graph)."""
    nchunks = tuple(int(c) for c in nchunks)
    assert len(nchunks) == SLOTS and all(1 <= c <= TV // P for c in nchunks)
    rows = [P * c for c in nchunks]
    row_off = np.cumsum([0] + rows).tolist()
    R = row_off[-1]
    w_all = [_piece_widths(n) for n in nchunks]
    W0 = P * w_all[0][0]
    boot_bytes = BOOT_BYTES
    # slot0 pieces 1.. : per-piece byte blobs [2,W] bf16 + [2,W] e4m3
    rest_w = w_all[0][1:]
    rest_off = np.cumsum([0] + [6 * P * w for w in rest_w]).tolist()

    nc = bacc.Bacc("TRN2", target_bir_lowering=False, debug=False,
                   enable_asserts=False)

    boot_d = nc.dram_tensor("boot", [P, boot_bytes], U8, kind="ExternalInput")
    vt0p0_d = nc.dram_tensor("vt0p0", [P, 6 * W0], U8, kind="ExternalInput")
    # slot0 pieces 1.. : piece1 alone, pieces 2.. merged (two dmas on the
    # scalar queue whose serialized transfers throttle V0's start until
    # the boot transfer is done - HBM contention control)
    vt0rest_d = nc.dram_tensor("vt0rest", [P, max(1, rest_off[-1])], U8,
                               kind="ExternalInput")
    # slots 1..: whole slot (bf16 half + fp8 half) as ONE byte blob each
    vts_d = nc.dram_tensor("vts", [P, max(1, 6 * (R - rows[0]))], U8,
                           kind="ExternalInput")
    # natural-layout value (context matmul): per-partition contiguous
    value_d = nc.dram_tensor("value", [P, R // P, DV], FP16,
                             kind="ExternalInput")
    out_d = nc.dram_tensor("out", [SLOTS, DV], F32, kind="ExternalOutput")

    with tile.TileContext(nc) as tc, ExitStack() as ctx:
        consts = ctx.enter_context(tc.tile_pool(name="consts", bufs=1))
        vt_pool = ctx.enter_context(tc.tile_pool(name="vt", bufs=3))
        v_pool = ctx.enter_context(tc.tile_pool(name="vsb", bufs=3))
        th_pool = ctx.enter_context(tc.tile_pool(name="th", bufs=9))
        sm_pool = ctx.enter_context(tc.tile_pool(name="sm", bufs=3))
        kps_pool = ctx.enter_context(
            tc.tile_pool(name="kps", bufs=4, space=bass.MemorySpace.PSUM))
        score_pool = ctx.enter_context(
            tc.tile_pool(name="scps", bufs=2, space=bass.MemorySpace.PSUM))
        ctx_pool = ctx.enter_context(
            tc.tile_pool(name="ctxps", bufs=1, space=bass.MemorySpace.PSUM))
        misc_pool = ctx.enter_context(
            tc.tile_pool(name="mps", bufs=1, space=bass.MemorySpace.PSUM))

        # ---- warmup constants FIRST so PE can start ramping ASAP ----
        ones_sq = consts.tile([P, P], BF16)
        nc.vector.memset(ones_sq[:], 1.0)
        wrhs = consts.tile([P, 256], BF16)
        nc.vector.memset(wrhs[:], 0.5)

        # PE warm-up emitted BEFORE any dma_start so it carries zero DMA
        # waits; sized to end when the boot transfer lands (~12.5us)
        for w in range(WARMUP):
            wu = kps_pool.tile([P, 256], F32, tag="kps", name=f"wu{w}")
            nc.tensor.matmul(wu[:], ones_sq[:], wrhs[:],
                             start=True, stop=True)

        # ---- the boot mega-DMA (gpsimd queue) ----
        boot = consts.tile([P, boot_bytes], U8)
        nc.gpsimd.dma_start(boot[:], boot_d.ap())
        Wkbf = boot[:, BOOT_WKBF:BOOT_WKBF + 2048].bitcast(BF16).rearrange(
            "p (j m) -> p j m", j=2)
        Wk8 = boot[:, BOOT_WK8:BOOT_WK8 + 1024].bitcast(FP8E4).rearrange(
            "p (i m) -> p i m", i=2)
        smalls = boot[:, BOOT_SMALLS:BOOT_SMALLS + 96].bitcast(F32)
        # slot0 piece0 travels FIRST on the scalar queue: with params-only
        # boot on gpsimd, both land ~12us (vs ~15us when p0 rode in boot
        # and p1/p23 shared HBM with the bigger transfer)
        vt0p0 = consts.tile([P, 6 * W0], U8)
        nc.scalar.dma_start(vt0p0[:], vt0p0_d.ap())
        vt0p0_bf = vt0p0[:, 0:4 * W0].bitcast(BF16).rearrange(
            "p (j t) -> p j t", j=2)
        vt0p0_8 = vt0p0[:, 4 * W0:6 * W0].bitcast(FP8E4).rearrange(
            "p (i t) -> p i t", i=2)

        # slot0 pieces 1..: piece 1 in its own tile/DMA, pieces 2.. merged
        # into one, both on the SCALAR queue.  Their serialized transfers
        # also keep V0 (same queue) from flooding HBM during boot.  The
        # tile scheduler hoists dispatch instructions, so only queue
        # ordering (not emission position) delays a transfer.
        vt0rest = []
        if len(rest_w) >= 1:
            vt0rest.append(consts.tile([P, rest_off[1]], U8, tag="vt0r1",
                                       name="vt0r1"))
        if len(rest_w) >= 2:
            vt0rest.append(consts.tile([P, rest_off[-1] - rest_off[1]], U8,
                                       tag="vt0r2", name="vt0r2"))
        nseg = len(vt0rest)

        def emit_rest(i):
            if i == 0 and nseg >= 1:
                nc.scalar.dma_start(vt0rest[0][:],
                                    vt0rest_d[:, 0:rest_off[1]])
            elif i == 1 and nseg >= 2:
                nc.scalar.dma_start(vt0rest[1][:],
                                    vt0rest_d[:, rest_off[1]:rest_off[-1]])

        def vt0_piece_aps(q):
            if q == 0:
                return vt0p0_bf, vt0p0_8
            W = P * w_all[0][q]
            if q == 1:
                t, o = vt0rest[0], 0
            else:
                t, o = vt0rest[1], rest_off[q - 1] - rest_off[1]
            bf = t[:, o:o + 4 * W].bitcast(BF16).rearrange(
                "p (j t) -> p j t", j=2)
            e4 = t[:, o + 4 * W:o + 6 * W].bitcast(FP8E4).rearrange(
                "p (i t) -> p i t", i=2)
            return bf, e4

        ones_col_f = consts.tile([P, 1], F32)
        nc.vector.memset(ones_col_f[:], 1.0)
        scaleT_sb = consts.tile([P, KC], FP16)
        nc.vector.tensor_copy(scaleT_sb[:], smalls[:, 16:20])
        orow_all = consts.tile([P, DV], F32)

        # per-slot selector columns for the 4-partial-row context reduce
        sel = []
        for s in range(SLOTS):
            ng = min(4, nchunks[s])
            sl = consts.tile([P, 1], FP16, tag=f"sel{s}")
            nc.vector.memset(sl[:], 0.0)
            for g in range(ng):
                nc.vector.memset(sl[32 * g:32 * g + 1, :], 1.0)
            sel.append(sl)

        # zero the context PSUM bank once: unwritten partitions must
        # read as finite 0.0 for the selector-matmul reduce
        z = ctx_pool.tile([P, 512], F32, tag="ctx4", name="ctxz")
        nc.vector.memset(z[:], 0.0)

        # masks: mask_s[p, 4q+c2] = (512q + p*w_q + c2 < len_s)
        mask = []
        for s in range(SLOTS):
            nch = nchunks[s]
            iota_i = consts.tile([P, nch], I32, tag=f"io{s}")
            for q, w in enumerate(w_all[s]):
                nc.gpsimd.iota(iota_i[:, 4 * q:4 * q + w], pattern=[[1, w]],
                               base=512 * q, channel_multiplier=w)
            iota_f = consts.tile([P, nch], F32, tag=f"iof{s}")
            nc.vector.tensor_copy(iota_f[:], iota_i[:])
            mk = consts.tile([P, nch], F32, tag=f"mask{s}")
            nc.vector.tensor_scalar(mk[:], iota_f[:], smalls[:, 20 + s:21 + s],
                                    None, op0=ALU.is_lt)
            mask.append(mk)

        # ---- per-slot emission, software-pipelined ----
        def emit_tail(s, score4, V_sb, use_act=False, pe_transpose=False):
            # at the kernel tail ACT is idle while DVE serializes the
            # softmax chain; route the two PSUM->SBUF copies to ACT there
            cp = nc.scalar.copy if use_act else nc.vector.tensor_copy
            nch = nchunks[s]
            widths = w_all[s]
            # scores [1, T'] live at partitions {0,32,64,96} of score4
            s4sb = sm_pool.tile([P, 512], F32, tag="s4sb", name=f"s4sb{s}")
            cp(s4sb[:], score4[:])
            if pe_transpose:
                # transpose via nch tiny PE matmuls into spare score4
                # columns; skips a ~2.2us sync-queue DMA on the exposed
                # final tail.  scT[p, 4q+c2] = s4sb[32q, p*w+c2].
                scT = score4[:, 512 - nch:512]
                for q, w in enumerate(widths):
                    cols = s4sb[32 * q:32 * q + 1, 0:P * w].rearrange(
                        "a (p c) -> a c p", p=P)
                    for c2 in range(w):
                        nc.tensor.matmul(scT[:, 4 * q + c2:4 * q + c2 + 1],
                                         cols[:, c2:c2 + 1, :],
                                         ones_col_f[32 * q:32 * q + 1, 0:1],
                                         start=True, stop=True,
                                         is_transpose=True,
                                         tile_position=(32 * q, 0))
            else:
                # all transpose DMAs stay on the sync queue: routing any
                # to gpsimd chains later gpsimd-queue consumers (and the
                # exp itself) behind the next slot's big VT transfer via
                # the conservative per-queue waits (measured +1.6us)
                scT = sm_pool.tile([P, nch], F32, tag="scT", name=f"scT{s}")
                for q, w in enumerate(widths):
                    src = s4sb[32 * q:32 * q + 1, 0:P * w].rearrange(
                        "a (p c) -> a p c", p=P)
                    nc.sync.dma_start(scT[:, 4 * q:4 * q + w], src)
            ex = sm_pool.tile([P, nch], F32, tag="ex", name=f"ex{s}")
            nc.scalar.activation(ex[:], scT[:], AF.Exp)
            exm = sm_pool.tile([P, nch], F32, tag="exm", name=f"exm{s}")
            nc.vector.tensor_tensor(exm[:], ex[:], mask[s][:], op=ALU.mult)
            rs = sm_pool.tile([P, 1], F32, tag="rs", name=f"rs{s}")
            nc.vector.reduce_sum(rs[:], exm[:], axis=mybir.AxisListType.X)
            attn = sm_pool.tile([P, nch], FP16, tag="attn", name=f"attn{s}")
            nc.vector.tensor_copy(attn[:], exm[:])
            stot = misc_pool.tile([1, 1], F32, tag="m", name=f"stot{s}")
            nc.tensor.matmul(stot[:], rs[:], ones_col_f[:],
                             start=True, stop=True)
            rcp = sm_pool.tile([1, 1], F32, tag="rcp", name=f"rcp{s}")
            nc.vector.reciprocal(rcp[:], stot[:])

            # context: M=1 matmuls, chunk c -> column-group (c mod 4)
            ctx4 = ctx_pool.tile([P, 512], F32, tag="ctx4", name=f"ctx4_{s}")
            for g in range(min(4, nch)):
                chain = list(range(g, nch, 4))
                for i, c in enumerate(chain):
                    nc.tensor.matmul(ctx4[32 * g:32 * g + 1, :],
                                     attn[:, c:c + 1], V_sb[:, c, :],
                                     start=(i == 0), stop=(i == len(chain) - 1),
                                     tile_position=(0, 32 * g))
            c4sb = sm_pool.tile([P, 512], FP16, tag="c4sb", name=f"c4sb{s}")
            cp(c4sb[:], ctx4[:])
            ctxred = misc_pool.tile([1, DV], F32, tag="m", name=f"cred{s}")
            nc.tensor.matmul(ctxred[:], sel[s][:], c4sb[:],
                             start=True, stop=True)
            # all slots write one persistent tile (rows 32s - engine ops
            # need 32-aligned base partitions); ONE out DMA at the end
            # (each dma_start costs ~2.2us of sync-queue time + teardown
            # barrier rounds scale with DMA count)
            nc.vector.tensor_scalar(orow_all[32 * s:32 * s + 1, :],
                                    ctxred[:], rcp[:], None, op0=ALU.mult)

        pending = None
        for s in range(SLOTS):
            nch = nchunks[s]
            widths = w_all[s]
            if s == 0:
                pieces = [vt0_piece_aps(q) for q in range(len(widths))]
                piece_c0 = [0] * len(widths)
            else:
                VTs = vt_pool.tile([P, 6 * rows[s]], U8, tag="vts",
                                   name=f"vts{s}")
                o6 = 6 * (row_off[s] - rows[0])
                nc.gpsimd.dma_start(VTs[:], vts_d[:, o6:o6 + 6 * rows[s]])
                pbf = VTs[:, 0:4 * rows[s]].bitcast(BF16).rearrange(
                    "p (j t) -> p j t", j=2)
                p8 = VTs[:, 4 * rows[s]:].bitcast(FP8E4).rearrange(
                    "p (i t) -> p i t", i=2)
                pieces = [(pbf, p8)] * len(widths)
                piece_c0 = [512 * q for q in range(len(widths))]
            score4 = score_pool.tile([P, 512], F32, tag="s4", name=f"s4{s}")

            ths = {}

            def kproj_piece(q, w):
                W = P * w
                c0 = piece_c0[q]
                pbf, p8 = pieces[q]
                for dc in range(KC):
                    kps = kps_pool.tile([P, W], F32, tag="kps",
                                        name=f"k{s}_{q}_{dc}")
                    for j in range(2):
                        nc.tensor.matmul(kps[:],
                                         Wkbf[:, j, dc * P:(dc + 1) * P],
                                         pbf[:, j, c0:c0 + W],
                                         start=(j == 0), stop=False)
                    nc.tensor.matmul(kps[:],
                                     Wk8[:, :, dc * P:(dc + 1) * P],
                                     p8[:, :, c0:c0 + W],
                                     start=False, stop=True,
                                     perf_mode=DR)
                    th = th_pool.tile([P, W], FP16, tag="th",
                                      name=f"th{s}_{q}_{dc}")
                    nc.scalar.activation(
                        th[:], kps[:], AF.Tanh,
                        bias=smalls[:, dc * SLOTS + s:dc * SLOTS + s + 1])
                    ths[(q, dc)] = th

            def score_piece(q, w):
                W = P * w
                for dc in range(KC):
                    nc.tensor.matmul(score4[32 * q:32 * q + 1, 0:W],
                                     scaleT_sb[:, dc:dc + 1],
                                     ths.pop((q, dc))[:],
                                     start=(dc == 0), stop=(dc == KC - 1),
                                     tile_position=(0, 32 * q))

            # V load on the scalar HWDGE queue, one contiguous dram run
            # per partition; consumed by tail(s) two pieces into slot
            # s+1.  Emitted late enough that slot0's remaining VT pieces
            # (same queue) aren't stuck behind it.
            V_sb = v_pool.tile([P, nch, DV], FP16, tag="vsb", name=f"vsb{s}")

            def emit_v_load():
                # scalar queue: its serialized order (p1, p23, V0..V3)
                # throttles V transfers against the boot/VT loads; sync
                # would let V0 flood HBM during boot and put V3 behind
                # scT2 (late for the exposed final tail)
                nc.scalar.dma_start(
                    V_sb[:].rearrange("p c d -> p (c d)"),
                    value_d[:, row_off[s] // P:row_off[s + 1] // P, :]
                    .rearrange("p c d -> p (c d)"))

            v_at = min(2, len(widths) - 1) if s == 0 else \
                min(1, len(widths) - 1)
            prev_piece = None
            tail_done = pending is None
            for q, w in enumerate(widths):
                kproj_piece(q, w)
                if s == 0 and q <= 1:
                    emit_rest(q)
                if q == v_at:
                    emit_v_load()
                if not tail_done and (q >= 1 or q == len(widths) - 1):
                    # defer the previous slot's softmax/context PE ops
                    # two pieces in: one piece doesn't cover the DVE/DMA
                    # softmax chain latency (PE stalls in-order)
                    # slot0 (4 pieces) uses the PE-transpose score path:
                    # its 4 per-piece scT DMAs cost 4x2.2us serialized on
                    # the sync queue and stalled PE 2.6us (measured)
                    emit_tail(*pending, use_act=(s == SLOTS - 1))
                    tail_done = True
                if prev_piece is not None:
                    score_piece(*prev_piece)
                prev_piece = (q, w)
            score_piece(*prev_piece)
            pending = (s, score4, V_sb)
        emit_tail(*pending, use_act=True, pe_transpose=True)
        nc.sync.dma_start(
            out_d[:],
            orow_all[:].rearrange("(s r) d -> s r d", r=32)[:, 0:1, :])

    nc.compile()
    return nc


_graph_cache = {}

# test-harness knobs (the grading path leaves these at defaults)
TRACE = False
TRACE_KWARGS = {}
LAST_RESULTS = None


def _get_graph(nchunks):
    key = tuple(nchunks)
    if key not in _graph_cache:
        _graph_cache[key] = build_graph(key)
    return _graph_cache[key]


def plan(value_lens):
    """Sort batches by length desc; rank r -> core r%8, slot r//8."""
    lens = np.asarray(value_lens, np.int64)
    order = np.argsort(-lens, kind="stable")
    nchunks = tuple(
        int(math.ceil(max(1, int(lens[order[s * N_CORES:(s + 1) * N_CORES]].max())) / P))
        for s in range(SLOTS))
    return order, nchunks


def _u8(a):
    return np.ascontiguousarray(a).view(np.uint8).reshape(a.shape[0], -1)


def prepare(query, value, value_lens, Wq, bq, Wk, bk, scale):
    query = np.ascontiguousarray(np.asarray(query, np.float32))
    value = np.ascontiguousarray(np.asarray(value, np.float32))
    lens = np.ascontiguousarray(np.asarray(value_lens, np.int32))
    Wq = np.ascontiguousarray(np.asarray(Wq, np.float32))
    bq = np.ascontiguousarray(np.asarray(bq, np.float32))
    Wk = np.ascontiguousarray(np.asarray(Wk, np.float32))
    bk = np.ascontiguousarray(np.asarray(bk, np.float32))
    scale = np.ascontiguousarray(np.asarray(scale, np.float32))

    order, nchunks = plan(lens)
    nc = _get_graph(nchunks)
    w_all = [_piece_widths(n) for n in nchunks]

    nat_idx = [_nat_index(nchunks[s]) for s in range(SLOTS)]
    scaleT = scale.reshape(KC, P).T.astype(np.float32)
    # q computed on host, with both biases folded in
    qhat = (query @ Wq + bq[None, :] + bk[None, :]).astype(np.float32)
    wkbf_u8 = _u8(np.stack([Wk[0:P], Wk[P:2 * P]], axis=1).astype(BF16_NP))
    wk8_u8 = _u8(np.stack([Wk[2 * P:3 * P], Wk[3 * P:4 * P]],
                          axis=1).astype(E4_NP))

    in_maps = []
    for c in range(N_CORES):
        bidx = [int(order[s * N_CORES + c]) for s in range(SLOTS)]
        vparts, vts_parts = [], []
        vt0_bf = vt0_8 = None
        for s in range(SLOTS):
            T = nchunks[s] * P
            vp = value[bidx[s], :T, :]  # [T, DV] f32 (T <= TV always)
            bf3 = vp[:, 0:256].T.reshape(2, P, T).transpose(1, 0, 2)
            e43 = vp[:, 256:512].T.reshape(2, P, T).transpose(1, 0, 2)
            if s == 0:
                vt0_bf = bf3.astype(BF16_NP)   # [P, 2, T]
                vt0_8 = e43.astype(E4_NP)
            else:
                vts_parts.append(_u8(np.ascontiguousarray(
                    bf3.astype(BF16_NP))))
                vts_parts.append(_u8(np.ascontiguousarray(
                    e43.astype(E4_NP))))
            vparts.append(vp[nat_idx[s]].astype(F16_NP))
        qt = qhat[bidx]  # [SLOTS, DQ] f32
        qt_cols = qt.T.reshape(KC, P, SLOTS).transpose(1, 0, 2).reshape(P, 16)
        lens_bc = np.broadcast_to(lens[bidx].astype(np.float32)[None, :],
                                  (P, SLOTS))
        smalls = np.concatenate(
            [qt_cols.astype(np.float32), scaleT, lens_bc],
            axis=1).astype(np.float32)

        widths0 = w_all[0]
        W0 = P * widths0[0]
        boot = np.concatenate([wkbf_u8, wk8_u8, _u8(smalls)], axis=1)
        vt0p0 = np.concatenate(
            [_u8(np.ascontiguousarray(vt0_bf[:, :, 0:W0])),
             _u8(np.ascontiguousarray(vt0_8[:, :, 0:W0]))], axis=1)
        rest = []
        c0 = W0
        for w in widths0[1:]:
            W = P * w
            rest.append(_u8(np.ascontiguousarray(vt0_bf[:, :, c0:c0 + W])))
            rest.append(_u8(np.ascontiguousarray(vt0_8[:, :, c0:c0 + W])))
            c0 += W
        vt0rest = (np.concatenate(rest, axis=1) if rest
                   else np.zeros((P, 1), np.uint8))

        # value: [P, R//P, DV] with partition-major packing so each
        # per-slot V load is ONE contiguous dram run per partition
        vall = np.concatenate(
            [vparts[s].reshape(P, nchunks[s], DV) for s in range(SLOTS)],
            axis=1)
        in_maps.append({
            "boot": np.ascontiguousarray(boot),
            "vt0p0": np.ascontiguousarray(vt0p0),
            "vt0rest": np.ascontiguousarray(vt0rest),
            "value": np.ascontiguousarray(vall),
            "vts": np.ascontiguousarray(np.concatenate(vts_parts, axis=1)),
        })
    return nc, in_maps, order, nchunks


def kernel(query, value, value_lens, Wq, bq, Wk, bk, scale):
    nc, in_maps, order, _ = prepare(query, value, value_lens,
                                    Wq, bq, Wk, bk, scale)

    res = bass_utils.run_bass_kernel_spmd(
        nc, in_maps, core_ids=list(range(N_CORES)), trace=TRACE,
        **TRACE_KWARGS)
    global LAST_RESULTS
    LAST_RESULTS = res

    out = np.zeros((B, 1, DV), np.float32)
    for c in range(N_CORES):
        o = res.results[c]["out"]
        for s in range(SLOTS):
            out[int(order[s * N_CORES + c]), 0, :] = o[s]
    return out
